# revision 20
# baseline (speedup 1.0000x reference)
"""BioRNN Trainium2 kernel (dev module).

Sharding: 16 time-windows (2 per core, ~63 output steps each), full
batch 64 per window. The leak (0.8/step) forgets initial state: a
32-step burn-in from h=0 reproduces the true state (window 0 pads with
zeros, exact). Per core: 2 windows x T_LOC=95 steps, emitted half-step
out of phase so one window's relu handoff hides under the other's
matmuls.

accum-q recurrence, v-part only in psum: q = 0.8^-j * v_t within a
Q=32 block (j = t % Q), v_t = h_{t-1} @ w_eff. Per window-step:
    1 inj matmul   identity @ dz_t (bank A only; N=128)
    16 W matmuls   r'_t @ w_eff (N=64)
where dz is the HOST-precomputed delta of z = x@w_in + noise + b_rec
for the bank-A half (r 0:256), pre-scaled by 0.8^-j, fp16, transposed.
Bank C's z half never enters psum: it rides in through a 2-input DVE
relu. The two relus run on DIFFERENT engines to halve the handoff
serialization:
    bank A (r 0:256):  ACT   r'a = relu(qA * c0)         (z in psum)
    bank C (r 256:512): DVE  r'c = relu(qC*c0 + z*c0)    (z from SBUF)
with c0 = 0.2*0.8^(jp-jn). Every Q steps each bank is re-injected at
true scale (ACT/DVE mul -> fp16 -> identity matmul, start=True).

h is NOT computed on device: h_t = 0.8 h_{t-1} + 0.8^jn r'_t is a
leaky integration the HOST does in f32 over the dumped r' stream
(more accurate than the old fp16 device ring). r' ring chunks DMA to
DRAM in native (p, t, v, m, b) fp16 layout; host un-transposes,
rescales by 0.8^jn(t), and lfilters to h.
"""

import os
import numpy as np
from contextlib import ExitStack

import concourse.bass as bass
import concourse.mybir as mybir
import concourse.tile as tile
from concourse import bacc
from concourse import dve_ops
from concourse.dve_spec import (
    Spec, Src0, Src1, C0, C1, relu as _dve_relu_expr, lower,
)
from concourse.dve_uop import DveOpSpec
from concourse.masks import make_identity


def _register_dve(name, body, ref, rd1=True):
    """Register a custom DVE op (idempotent)."""
    for o in dve_ops.OPS:
        if o.name == name:
            return o
    opcode = max(dve_ops._SUB_OPCODE_FOR_NAME.values()) + 1
    assert opcode < 0x20
    dve_ops._SUB_OPCODE_FOR_NAME[name] = opcode
    spec = Spec(body=body, reference=ref)
    shas = {}
    for ver in ("v3", "v4"):
        s = DveOpSpec(name=name, opcode=opcode, uops=lower(spec, ver=ver),
                      rd1_en=rd1)
        shas[ver] = s.sha(ver)
    op = dve_ops.DveOp(name, spec, subdim=False, uops_sha=shas)
    dve_ops.OPS.append(op)
    dve_ops.CUSTOM_DVE_SPECS[name] = spec
    return op


def _f32(a):
    return a.astype(np.float32).reshape(a.shape[0], -1)


def _ref_relu2(in0, in1, c0, c1, c2):
    s = np.maximum(np.nan_to_num(_f32(in0) * c0 + _f32(in1) * c1,
                                 nan=0.0, posinf=np.inf, neginf=-np.inf), 0)
    return s.reshape(in0.shape)


RELU2 = _register_dve("RELU2_BIO", _dve_relu_expr(Src0 * C0 + Src1 * C1),
                      _ref_relu2, rd1=True)

F32 = mybir.dt.float32
F16 = mybir.dt.float16
RELU_FN = mybir.ActivationFunctionType.Relu

R = 512          # n_rec
NIN = 128        # n_in
RC = 4           # r chunks
B = 64           # batch (full, per window)
N_CORES = 8
NV = 2           # windows per core
NWIN = N_CORES * NV
T_FULL = 1000
BURN = int(os.environ.get("BIO_BURN", "48"))  # burn-in steps
WSTARTS = [(w * T_FULL) // NWIN for w in range(NWIN + 1)]
WLEN = max(WSTARTS[w + 1] - WSTARTS[w] for w in range(NWIN))  # 63
T_LOC = WLEN + BURN       # local steps per window
ALPHA = 0.2
LEAK = 1.0 - ALPHA
Q = 40                    # accum-q rescale block (0.8^-39 ~ 6e3: fp16-safe)
QOFF = (0, Q // 2)        # per-window frame offset: restarts never coincide
U = 48                    # r' ring steps
CPS = NV * 256            # ring cols per step-slot
ZCH = 8                   # z-stream DMA chunk (steps)
ZLEAD = 16                # z chunks DMA'd this many steps ahead
OCH = 8                   # out-dump chunk (steps)
OSKIP = BURN - 24         # dump starts here: 24 pre-output steps suffice
                          # for host h-integration (0.8^24 ~ 5e-3 missing)


def build_nc(T=T_LOC):
    """Build the per-core Bass program."""
    nc = bacc.Bacc()

    # host-precomputed pre-scaled z streams, fp16, transposed layout
    # [p, v, t, c]: c = m*64+b; c 0:128 = delta-enc (bank A), 128:256 = raw
    z_d = nc.dram_tensor("z16", [128, NV, T, 256], F16,
                         kind="ExternalInput").ap()
    w_d = nc.dram_tensor("w16", [R, R], F16, kind="ExternalInput").ap()
    # raw r' dump (steps OSKIP..T): [p, t, v*256 + m*64 + b] fp16;
    # host integrates h
    o_d = nc.dram_tensor("outT16", [128, T - OSKIP, CPS], F16,
                         kind="ExternalOutput").ap()

    with tile.TileContext(nc) as tc, ExitStack() as ctx:
        const = ctx.enter_context(tc.tile_pool(name="const", bufs=1))
        big = ctx.enter_context(tc.tile_pool(name="big", bufs=1))
        sp = ctx.enter_context(tc.tile_pool(name="sp", bufs=2))

        ident16 = const.tile([128, 128], F16)
        make_identity(nc, ident16[:, :])
        zero16 = const.tile([128, 128], F16)
        nc.vector.memset(zero16[:, :], 0.0)

        w16 = big.tile([128, RC * R], F16)
        zt = big.tile([128, NV * T * 256], F16)   # z streams, full resident
        rr = big.tile([128, U * CPS], F16)        # r' ring

        pools = [ctx.enter_context(
            tc.tile_pool(name=f"ps{n}", bufs=1, space="PSUM"))
            for n in ("a0", "c0", "a1", "c1")]
        pst = [p.tile([128, 512], F32, name=f"ps{i}", tag=f"ps{i}")
               for i, p in enumerate(pools)]
        psA = [pst[0], pst[2]]
        psC = [pst[1], pst[3]]

        # ---- DMA helpers: in-triggers must never sit behind sem-gated
        # out-triggers (head-of-line), so the streams get separate queues
        def emit_in(v, t0, t1):
            t1 = min(t1, T)
            if t0 >= t1:
                return
            nc.gpsimd.dma_start(
                out=zt[:, (v * T + t0) * 256:(v * T + t1) * 256]
                .rearrange("p (t c) -> p t c", c=256),
                in_=z_d[:, v, t0:t1, :])

        def emit_out(a, e):
            s0 = (a % U) * CPS
            nc.sync.dma_start(
                out=o_d[:, a - OSKIP:e - OSKIP, :],
                in_=rr[:, s0:s0 + (e - a) * CPS]
                .rearrange("p (t c) -> p t c", c=CPS))

        # startup: first chunks + single-trigger weight load
        emit_in(0, 0, 8)
        nc.sync.dma_start(
            out=w16[:, :].rearrange("p (k c) -> p k c", c=R),
            in_=w_d.rearrange("(k p) c -> p k c", p=128))
        emit_in(1, 0, 8)
        for t0 in range(ZCH, min(ZLEAD, T), ZCH):
            for v in range(NV):
                emit_in(v, t0, t0 + ZCH)

        mm = nc.tensor.matmul

        def block(v, t):
            pa, pc = psA[v], psC[v]
            base = (v * T + t) * 256
            zA = zt[:, base:base + 128]
            zC = zt[:, base + 128:base + 256]
            so = (t % U) * CPS + v * 256
            outA = rr[:, so:so + 128]
            outC = rr[:, so + 128:so + 256]
            pr = ((t - 1) % U) * CPS + v * 256

            def rk(k):
                return rr[:, pr + k * 64:pr + (k + 1) * 64]

            if t == 0:
                mm(pa[:, :128], lhsT=ident16[:, :], rhs=zA,
                   start=True, stop=True, skip_group_check=True)
                mm(pc[:, :128], lhsT=ident16[:, :], rhs=zero16[:, :],
                   start=True, stop=True, skip_group_check=True)
            else:
                if (t + QOFF[v]) % Q == 0:
                    # re-inject q at true scale (q := 0.8^Q * q)
                    s16a = sp.tile([128, 128], F16, tag=f"s16a{v}")
                    s16c = sp.tile([128, 128], F16, tag=f"s16c{v}")
                    nc.scalar.mul(out=s16a[:, :], in_=pa[:, :128],
                                  mul=float(LEAK ** Q))
                    nc.vector.tensor_scalar_mul(s16c[:, :], pc[:, :128],
                                                float(LEAK ** Q))
                    mm(pa[:, :128], lhsT=ident16[:, :], rhs=s16a[:, :],
                       start=True, stop=False, skip_group_check=True)
                    mm(pc[:, :128], lhsT=ident16[:, :], rhs=s16c[:, :],
                       start=True, stop=False, skip_group_check=True)

                def kmm(m, k, stop=False):
                    ps = pa if m < 2 else pc
                    off = (m % 2) * 64
                    mm(ps[:, off:off + 64],
                       lhsT=w16[:, k * R + m * 128:k * R + (m + 1) * 128],
                       rhs=rk(k), start=False, stop=stop,
                       skip_group_check=True)

                mm(pa[:, :128], lhsT=ident16[:, :], rhs=zA,
                   start=False, stop=False, skip_group_check=True)
                kmm(0, 0); kmm(1, 0); kmm(0, 1); kmm(1, 1)
                kmm(2, 0); kmm(3, 0); kmm(2, 1); kmm(3, 1)
                kmm(0, 2); kmm(1, 2); kmm(0, 3); kmm(1, 3, stop=True)
                kmm(2, 2); kmm(3, 2); kmm(2, 3); kmm(3, 3, stop=True)

            jp = (t + QOFF[v]) % Q
            jn = (t + 1 + QOFF[v]) % Q
            c0 = float(ALPHA * LEAK ** (jp - jn))
            nc.scalar.activation(out=outA, in_=pa[:, :128], func=RELU_FN,
                                 scale=c0)
            nc.vector._custom_dve(RELU2, out=outC, in0=pc[:, :128],
                                  in1=zC, s0=c0, s1=c0)

        for t in range(T):
            tp = t + ZLEAD
            if tp % ZCH == 0 and tp < T:
                emit_in(0, tp, tp + ZCH)
                emit_in(1, tp, tp + ZCH)
            block(0, t)
            block(1, t)
            if (t + 1 - OSKIP) % OCH == 0 or t == T - 1:
                a = OSKIP + ((t - OSKIP) // OCH) * OCH
                if a >= OSKIP:
                    emit_out(a, t + 1)

    nc.compile()
    return nc


def host_prep(x, w_in, w_rec, b_rec, ei_mask, autapse_mask, noise):
    """Host-side weight prep + window shard + pre-scaled fp16 z streams.

    z = x@w_in + noise + b_rec. Bank-A half (r 0:256): delta-encoded
    (z_t - 0.8 z_{t-1}) for psum injection. Bank-C half (r 256:512):
    raw (added at the DVE relu). Both scaled by 0.8^-(t % Q).
    """
    ei = np.diagonal(np.asarray(ei_mask)).astype(np.float32)
    w_eff = ei[:, None] * (np.asarray(w_rec) * np.asarray(autapse_mask))
    w16 = w_eff.astype(np.float16)
    x = np.asarray(x, dtype=np.float32)
    z = (x.reshape(-1, NIN) @ np.asarray(w_in, dtype=np.float32)).reshape(
        B, T_FULL, R)
    z += np.asarray(noise, dtype=np.float32)
    z += np.asarray(b_rec, np.float32)
    in_maps = []
    for c in range(N_CORES):
        zwins = []
        for v in range(NV):
            jscale = (LEAK ** -((np.arange(T_LOC) + QOFF[v]) % Q)
                      ).astype(np.float32)
            w = NV * c + v
            t0 = WSTARTS[w] - BURN
            zp = np.zeros((B, T_LOC, R), np.float32)
            s = max(t0, 0)
            zp[:, s - t0:] = z[:, s:t0 + T_LOC]
            # [p, t, m, b]
            zt4 = zp.reshape(B, T_LOC, RC, 128).transpose(3, 1, 2, 0)
            dA = zt4[:, :, 0:2, :].copy()
            dA[:, 1:] -= LEAK * dA[:, :-1].copy()
            dA *= jscale[None, :, None, None]
            zC = zt4[:, :, 2:4, :] * jscale[None, :, None, None]
            zwins.append(np.concatenate(
                [dA.reshape(128, T_LOC, 128), zC.reshape(128, T_LOC, 128)],
                axis=2))
        z16 = np.ascontiguousarray(
            np.stack(zwins, axis=1).astype(np.float16))
        in_maps.append({"z16": z16, "w16": w16})
    return in_maps, w_eff.astype(np.float32)


def _integrate(dump):
    """dump: [128, T_LOC-OSKIP, CPS] fp16 per core (steps OSKIP..T_LOC)
    -> list of NV h arrays (B, T_LOC-OSKIP, R) f32 via host leaky
    integration from zero at OSKIP."""
    td = dump.shape[1]
    hs = []
    for v in range(NV):
        jn = ((np.arange(td) + OSKIP + 1 + QOFF[v]) % Q).astype(np.float32)
        sc = (LEAK ** jn).astype(np.float32)
        rp = dump[:, :, v * 256:(v + 1) * 256].astype(np.float32)
        # [p, t, m, b] -> [b, t, r]
        ar = rp.reshape(128, td, RC, B).transpose(3, 1, 2, 0).reshape(
            B, td, R)
        ar *= sc[None, :, None]
        h = np.empty_like(ar)
        acc = np.zeros((B, R), np.float32)
        for t in range(td):
            acc = LEAK * acc + ar[:, t]
            h[:, t] = acc
        hs.append(h)
    return hs


def reference_np(x, w_in, b_rec, w_eff, noise, T=None):
    """Numpy reference for dev checks (f32)."""
    x = np.asarray(x, np.float32)
    if T is None:
        T = x.shape[1]
    z = np.einsum("bti,ir->btr", x[:, :T], np.asarray(w_in)) \
        + np.asarray(noise)[:, :T] + np.asarray(b_rec)
    h = np.zeros((x.shape[0], w_eff.shape[0]), np.float32)
    outs = []
    for t in range(T):
        pre = z[:, t] + h @ w_eff
        h = LEAK * h + ALPHA * np.maximum(pre, 0.0)
        outs.append(h.copy())
    return np.stack(outs, axis=1)


# ---------------------------------------------------------------------------
# harness entry point
# ---------------------------------------------------------------------------
_NC_CACHE = {}


def kernel(x, w_in, w_rec, b_rec, ei_mask, autapse_mask, noise):
    from concourse.bass_utils import run_bass_kernel_spmd

    x = np.asarray(x)
    T = x.shape[1]
    in_maps, _ = host_prep(x, w_in, w_rec, b_rec, ei_mask, autapse_mask, noise)
    if T not in _NC_CACHE:
        _NC_CACHE[T] = build_nc()
    nc = _NC_CACHE[T]
    res = run_bass_kernel_spmd(nc, in_maps, core_ids=list(range(N_CORES)))
    out = np.empty((x.shape[0], T, R), np.float32)
    for c in range(N_CORES):
        hs = _integrate(res.results[c]["outT16"])
        for v in range(NV):
            w = NV * c + v
            a, e = WSTARTS[w], WSTARTS[w + 1]
            b0 = BURN - OSKIP
            out[:, a:e] = hs[v][:, b0:b0 + (e - a)]
    return out


# revision 26
# speedup vs baseline: 1.0924x; 1.0924x over previous
"""BioRNN Trainium2 kernel (dev module).

Sharding: 16 time-windows (2 per core, ~63 output steps each), full
batch 64 per window. The leak (0.8/step) forgets initial state: a
32-step burn-in from h=0 reproduces the true state (window 0 pads with
zeros, exact). Per core: 2 windows x T_LOC=95 steps, emitted half-step
out of phase so one window's relu handoff hides under the other's
matmuls.

accum-q recurrence, v-part only in psum: q = 0.8^-j * v_t within a
Q=32 block (j = t % Q), v_t = h_{t-1} @ w_eff. Per window-step:
    1 inj matmul   identity @ dz_t (bank A only; N=128)
    16 W matmuls   r'_t @ w_eff (N=64)
where dz is the HOST-precomputed delta of z = x@w_in + noise + b_rec
for the bank-A half (r 0:256), pre-scaled by 0.8^-j, fp16, transposed.
Bank C's z half never enters psum: it rides in through a 2-input DVE
relu. The two relus run on DIFFERENT engines to halve the handoff
serialization:
    bank A (r 0:256):  ACT   r'a = relu(qA * c0)         (z in psum)
    bank C (r 256:512): DVE  r'c = relu(qC*c0 + z*c0)    (z from SBUF)
with c0 = 0.2*0.8^(jp-jn). Every Q steps each bank is re-injected at
true scale (ACT/DVE mul -> fp16 -> identity matmul, start=True).

h is NOT computed on device: h_t = 0.8 h_{t-1} + 0.8^jn r'_t is a
leaky integration the HOST does in f32 over the dumped r' stream
(more accurate than the old fp16 device ring). r' ring chunks DMA to
DRAM in native (p, t, v, m, b) fp16 layout; host un-transposes,
rescales by 0.8^jn(t), and lfilters to h.
"""

import os
import numpy as np
from contextlib import ExitStack

import concourse.bass as bass
import concourse.mybir as mybir
import concourse.tile as tile
from concourse import bacc
from concourse import dve_ops
from concourse.dve_spec import (
    Spec, Src0, Src1, C0, C1, relu as _dve_relu_expr, lower,
)
from concourse.dve_uop import DveOpSpec
from concourse.masks import make_identity


def _register_dve(name, body, ref, rd1=True):
    """Register a custom DVE op (idempotent)."""
    for o in dve_ops.OPS:
        if o.name == name:
            return o
    opcode = max(dve_ops._SUB_OPCODE_FOR_NAME.values()) + 1
    assert opcode < 0x20
    dve_ops._SUB_OPCODE_FOR_NAME[name] = opcode
    spec = Spec(body=body, reference=ref)
    shas = {}
    for ver in ("v3", "v4"):
        s = DveOpSpec(name=name, opcode=opcode, uops=lower(spec, ver=ver),
                      rd1_en=rd1)
        shas[ver] = s.sha(ver)
    op = dve_ops.DveOp(name, spec, subdim=False, uops_sha=shas)
    dve_ops.OPS.append(op)
    dve_ops.CUSTOM_DVE_SPECS[name] = spec
    return op


def _f32(a):
    return a.astype(np.float32).reshape(a.shape[0], -1)


def _ref_relu2(in0, in1, c0, c1, c2):
    s = np.maximum(np.nan_to_num(_f32(in0) * c0 + _f32(in1) * c1,
                                 nan=0.0, posinf=np.inf, neginf=-np.inf), 0)
    return s.reshape(in0.shape)


RELU2 = _register_dve("RELU2_BIO", _dve_relu_expr(Src0 * C0 + Src1 * C1),
                      _ref_relu2, rd1=True)

F32 = mybir.dt.float32
F16 = mybir.dt.float16
RELU_FN = mybir.ActivationFunctionType.Relu

R = 512          # n_rec
NIN = 128        # n_in
RC = 4           # r chunks
B = 64           # batch (full, per window)
N_CORES = 8
NV = 2           # windows per core
NWIN = N_CORES * NV
T_FULL = 1000
BURN = int(os.environ.get("BIO_BURN", "48"))  # burn-in steps
WSTARTS = [(w * T_FULL) // NWIN for w in range(NWIN + 1)]
WLEN = max(WSTARTS[w + 1] - WSTARTS[w] for w in range(NWIN))  # 63
T_LOC = WLEN + BURN       # local steps per window
ALPHA = 0.2
LEAK = 1.0 - ALPHA
Q = 40                    # accum-q rescale block (0.8^-39 ~ 6e3: fp16-safe)
QOFF = (0, Q // 2)        # per-window frame offset: restarts never coincide
U = 64                    # r' ring steps
CPS = NV * 256            # ring cols per step-slot
ZR = 24                   # z-stream ring steps: the ring WAR paces the
                          # in-DMA against compute (else gpsimd floods
                          # HBM with the whole stream and PE inst-fetch
                          # starves)
ZCH = 8                   # z-stream DMA chunk (steps)
ZLEAD = 16                # z chunks DMA'd this many steps ahead
OCH = 8                   # out-dump chunk (steps)
OSKIP = BURN - 24         # dump starts here: 24 pre-output steps suffice
                          # for host h-integration (0.8^24 ~ 5e-3 missing)


def build_nc(T=T_LOC):
    """Build the per-core Bass program."""
    nc = bacc.Bacc()

    # host-precomputed pre-scaled z streams, fp16, transposed layout
    # [p, v, t, c]: c = m*64+b; c 0:128 = delta-enc (bank A), 128:256 = raw
    z_d = nc.dram_tensor("z16", [128, NV, T, 256], F16,
                         kind="ExternalInput").ap()
    w_d = nc.dram_tensor("w16", [R, R], F16, kind="ExternalInput").ap()
    # raw r' dump (steps OSKIP..T): [p, t, v*256 + m*64 + b] fp16;
    # host integrates h
    o_d = nc.dram_tensor("outT16", [128, T - OSKIP, CPS], F16,
                         kind="ExternalOutput").ap()

    with tile.TileContext(nc) as tc, ExitStack() as ctx:
        const = ctx.enter_context(tc.tile_pool(name="const", bufs=1))
        big = ctx.enter_context(tc.tile_pool(name="big", bufs=1))
        sp = ctx.enter_context(tc.tile_pool(name="sp", bufs=2))

        ident16 = const.tile([128, 128], F16)
        make_identity(nc, ident16[:, :])
        zero16 = const.tile([128, 128], F16)
        nc.vector.memset(zero16[:, :], 0.0)

        w16 = big.tile([128, RC * R], F16)
        zt = big.tile([128, NV * ZR * 256], F16)  # z stream ring
        rr = big.tile([128, U * CPS], F16)        # r' ring

        pools = [ctx.enter_context(
            tc.tile_pool(name=f"ps{n}", bufs=1, space="PSUM"))
            for n in ("a0", "c0", "a1", "c1")]
        pst = [p.tile([128, 512], F32, name=f"ps{i}", tag=f"ps{i}")
               for i, p in enumerate(pools)]
        psA = [pst[0], pst[2]]
        psC = [pst[1], pst[3]]

        # ---- DMA helpers: in-triggers must never sit behind sem-gated
        # out-triggers (head-of-line), so the streams get separate queues
        def emit_in(v, t0, t1):
            t1 = min(t1, T)
            if t0 >= t1:
                return
            s0 = (v * ZR + t0 % ZR) * 256
            nc.gpsimd.dma_start(
                out=zt[:, s0:s0 + (t1 - t0) * 256]
                .rearrange("p (t c) -> p t c", c=256),
                in_=z_d[:, v, t0:t1, :])

        def emit_out(a, e):
            s0 = (a % U) * CPS
            nc.sync.dma_start(
                out=o_d[:, a - OSKIP:e - OSKIP, :],
                in_=rr[:, s0:s0 + (e - a) * CPS]
                .rearrange("p (t c) -> p t c", c=CPS))

        # startup: first chunks + single-trigger weight load
        emit_in(0, 0, 8)
        nc.sync.dma_start(
            out=w16[:, :].rearrange("p (k c) -> p k c", c=R),
            in_=w_d.rearrange("(k p) c -> p k c", p=128))
        emit_in(1, 0, 8)
        for t0 in range(ZCH, min(ZR, T), ZCH):
            for v in range(NV):
                emit_in(v, t0, t0 + ZCH)

        mm = nc.tensor.matmul

        def block(v, t):
            pa, pc = psA[v], psC[v]
            base = (v * ZR + t % ZR) * 256
            zA = zt[:, base:base + 128]
            zC = zt[:, base + 128:base + 256]
            so = (t % U) * CPS + v * 256
            outA = rr[:, so:so + 128]
            outC = rr[:, so + 128:so + 256]
            pr = ((t - 1) % U) * CPS + v * 256

            def rk(k):
                return rr[:, pr + k * 64:pr + (k + 1) * 64]

            if t == 0:
                mm(pa[:, :128], lhsT=ident16[:, :], rhs=zA,
                   start=True, stop=True, skip_group_check=True)
                mm(pc[:, :128], lhsT=ident16[:, :], rhs=zero16[:, :],
                   start=True, stop=True, skip_group_check=True)
            else:
                if (t + QOFF[v]) % Q == 0:
                    # re-inject q at true scale (q := 0.8^Q * q)
                    s16a = sp.tile([128, 128], F16, tag=f"s16a{v}")
                    s16c = sp.tile([128, 128], F16, tag=f"s16c{v}")
                    nc.scalar.mul(out=s16a[:, :], in_=pa[:, :128],
                                  mul=float(LEAK ** Q))
                    nc.vector.tensor_scalar_mul(s16c[:, :], pc[:, :128],
                                                float(LEAK ** Q))
                    mm(pa[:, :128], lhsT=ident16[:, :], rhs=s16a[:, :],
                       start=True, stop=False, skip_group_check=True)
                    mm(pc[:, :128], lhsT=ident16[:, :], rhs=s16c[:, :],
                       start=True, stop=False, skip_group_check=True)

                def kmm(m, k, stop=False):
                    ps = pa if m < 2 else pc
                    off = (m % 2) * 64
                    mm(ps[:, off:off + 64],
                       lhsT=w16[:, k * R + m * 128:k * R + (m + 1) * 128],
                       rhs=rk(k), start=False, stop=stop,
                       skip_group_check=True)

                mm(pa[:, :128], lhsT=ident16[:, :], rhs=zA,
                   start=False, stop=False, skip_group_check=True)
                kmm(0, 0); kmm(1, 0); kmm(0, 1); kmm(1, 1)
                kmm(2, 0); kmm(3, 0); kmm(2, 1); kmm(3, 1)
                kmm(0, 2); kmm(1, 2); kmm(0, 3); kmm(1, 3, stop=True)
                kmm(2, 2); kmm(3, 2); kmm(2, 3); kmm(3, 3, stop=True)

            jp = (t + QOFF[v]) % Q
            jn = (t + 1 + QOFF[v]) % Q
            c0 = float(ALPHA * LEAK ** (jp - jn))
            nc.scalar.activation(out=outA, in_=pa[:, :128], func=RELU_FN,
                                 scale=c0)
            nc.vector._custom_dve(RELU2, out=outC, in0=pc[:, :128],
                                  in1=zC, s0=c0, s1=c0)

        for t in range(T):
            tp = t + ZLEAD
            if tp % ZCH == 0 and ZR <= tp < T:
                emit_in(0, tp, tp + ZCH)
                emit_in(1, tp, tp + ZCH)
            block(0, t)
            block(1, t)
            if (t + 1 - OSKIP) % OCH == 0 or t == T - 1:
                a = OSKIP + ((t - OSKIP) // OCH) * OCH
                if a >= OSKIP:
                    emit_out(a, t + 1)

    nc.compile()
    return nc


def host_prep(x, w_in, w_rec, b_rec, ei_mask, autapse_mask, noise):
    """Host-side weight prep + window shard + pre-scaled fp16 z streams.

    z = x@w_in + noise + b_rec. Bank-A half (r 0:256): delta-encoded
    (z_t - 0.8 z_{t-1}) for psum injection. Bank-C half (r 256:512):
    raw (added at the DVE relu). Both scaled by 0.8^-(t % Q).
    """
    ei = np.diagonal(np.asarray(ei_mask)).astype(np.float32)
    w_eff = ei[:, None] * (np.asarray(w_rec) * np.asarray(autapse_mask))
    w16 = w_eff.astype(np.float16)
    x = np.asarray(x, dtype=np.float32)
    z = (x.reshape(-1, NIN) @ np.asarray(w_in, dtype=np.float32)).reshape(
        B, T_FULL, R)
    z += np.asarray(noise, dtype=np.float32)
    z += np.asarray(b_rec, np.float32)
    in_maps = []
    for c in range(N_CORES):
        zwins = []
        for v in range(NV):
            jscale = (LEAK ** -((np.arange(T_LOC) + QOFF[v]) % Q)
                      ).astype(np.float32)
            w = NV * c + v
            t0 = WSTARTS[w] - BURN
            zp = np.zeros((B, T_LOC, R), np.float32)
            s = max(t0, 0)
            zp[:, s - t0:] = z[:, s:t0 + T_LOC]
            # [p, t, m, b]
            zt4 = zp.reshape(B, T_LOC, RC, 128).transpose(3, 1, 2, 0)
            dA = zt4[:, :, 0:2, :].copy()
            dA[:, 1:] -= LEAK * dA[:, :-1].copy()
            dA *= jscale[None, :, None, None]
            zC = zt4[:, :, 2:4, :] * jscale[None, :, None, None]
            zwins.append(np.concatenate(
                [dA.reshape(128, T_LOC, 128), zC.reshape(128, T_LOC, 128)],
                axis=2))
        z16 = np.ascontiguousarray(
            np.stack(zwins, axis=1).astype(np.float16))
        in_maps.append({"z16": z16, "w16": w16})
    return in_maps, w_eff.astype(np.float32)


def _integrate(dump):
    """dump: [128, T_LOC-OSKIP, CPS] fp16 per core (steps OSKIP..T_LOC)
    -> list of NV h arrays (B, T_LOC-OSKIP, R) f32 via host leaky
    integration from zero at OSKIP."""
    td = dump.shape[1]
    hs = []
    for v in range(NV):
        jn = ((np.arange(td) + OSKIP + 1 + QOFF[v]) % Q).astype(np.float32)
        sc = (LEAK ** jn).astype(np.float32)
        rp = dump[:, :, v * 256:(v + 1) * 256].astype(np.float32)
        # [p, t, m, b] -> [b, t, r]
        ar = rp.reshape(128, td, RC, B).transpose(3, 1, 2, 0).reshape(
            B, td, R)
        ar *= sc[None, :, None]
        h = np.empty_like(ar)
        acc = np.zeros((B, R), np.float32)
        for t in range(td):
            acc = LEAK * acc + ar[:, t]
            h[:, t] = acc
        hs.append(h)
    return hs


def reference_np(x, w_in, b_rec, w_eff, noise, T=None):
    """Numpy reference for dev checks (f32)."""
    x = np.asarray(x, np.float32)
    if T is None:
        T = x.shape[1]
    z = np.einsum("bti,ir->btr", x[:, :T], np.asarray(w_in)) \
        + np.asarray(noise)[:, :T] + np.asarray(b_rec)
    h = np.zeros((x.shape[0], w_eff.shape[0]), np.float32)
    outs = []
    for t in range(T):
        pre = z[:, t] + h @ w_eff
        h = LEAK * h + ALPHA * np.maximum(pre, 0.0)
        outs.append(h.copy())
    return np.stack(outs, axis=1)


# ---------------------------------------------------------------------------
# harness entry point
# ---------------------------------------------------------------------------
_NC_CACHE = {}


def kernel(x, w_in, w_rec, b_rec, ei_mask, autapse_mask, noise):
    from concourse.bass_utils import run_bass_kernel_spmd

    x = np.asarray(x)
    T = x.shape[1]
    in_maps, _ = host_prep(x, w_in, w_rec, b_rec, ei_mask, autapse_mask, noise)
    if T not in _NC_CACHE:
        _NC_CACHE[T] = build_nc()
    nc = _NC_CACHE[T]
    res = run_bass_kernel_spmd(nc, in_maps, core_ids=list(range(N_CORES)))
    out = np.empty((x.shape[0], T, R), np.float32)
    for c in range(N_CORES):
        hs = _integrate(res.results[c]["outT16"])
        for v in range(NV):
            w = NV * c + v
            a, e = WSTARTS[w], WSTARTS[w + 1]
            b0 = BURN - OSKIP
            out[:, a:e] = hs[v][:, b0:b0 + (e - a)]
    return out


# revision 28
# speedup vs baseline: 1.1596x; 1.0615x over previous
"""BioRNN Trainium2 kernel (dev module).

Sharding: 16 time-windows (2 per core, ~63 output steps each), full
batch 64 per window. The leak (0.8/step) forgets initial state: a
32-step burn-in from h=0 reproduces the true state (window 0 pads with
zeros, exact). Per core: 2 windows x T_LOC=95 steps, emitted half-step
out of phase so one window's relu handoff hides under the other's
matmuls.

accum-q recurrence, v-part only in psum: q = 0.8^-j * v_t within a
Q=32 block (j = t % Q), v_t = h_{t-1} @ w_eff. Per window-step:
    1 inj matmul   identity @ dz_t (bank A only; N=128)
    16 W matmuls   r'_t @ w_eff (N=64)
where dz is the HOST-precomputed delta of z = x@w_in + noise + b_rec
for the bank-A half (r 0:256), pre-scaled by 0.8^-j, fp16, transposed.
Bank C's z half never enters psum: it rides in through a 2-input DVE
relu. The two relus run on DIFFERENT engines to halve the handoff
serialization:
    bank A (r 0:256):  ACT   r'a = relu(qA * c0)         (z in psum)
    bank C (r 256:512): DVE  r'c = relu(qC*c0 + z*c0)    (z from SBUF)
with c0 = 0.2*0.8^(jp-jn). Every Q steps each bank is re-injected at
true scale (ACT/DVE mul -> fp16 -> identity matmul, start=True).

h is NOT computed on device: h_t = 0.8 h_{t-1} + 0.8^jn r'_t is a
leaky integration the HOST does in f32 over the dumped r' stream
(more accurate than the old fp16 device ring). r' ring chunks DMA to
DRAM in native (p, t, v, m, b) fp16 layout; host un-transposes,
rescales by 0.8^jn(t), and lfilters to h.
"""

import os
import numpy as np
from contextlib import ExitStack

import concourse.bass as bass
import concourse.mybir as mybir
import concourse.tile as tile
from concourse import bacc
from concourse import dve_ops
from concourse.dve_spec import (
    Spec, Src0, Src1, C0, C1, relu as _dve_relu_expr, lower,
)
from concourse.dve_uop import DveOpSpec
from concourse.masks import make_identity


def _register_dve(name, body, ref, rd1=True):
    """Register a custom DVE op (idempotent)."""
    for o in dve_ops.OPS:
        if o.name == name:
            return o
    opcode = max(dve_ops._SUB_OPCODE_FOR_NAME.values()) + 1
    assert opcode < 0x20
    dve_ops._SUB_OPCODE_FOR_NAME[name] = opcode
    spec = Spec(body=body, reference=ref)
    shas = {}
    for ver in ("v3", "v4"):
        s = DveOpSpec(name=name, opcode=opcode, uops=lower(spec, ver=ver),
                      rd1_en=rd1)
        shas[ver] = s.sha(ver)
    op = dve_ops.DveOp(name, spec, subdim=False, uops_sha=shas)
    dve_ops.OPS.append(op)
    dve_ops.CUSTOM_DVE_SPECS[name] = spec
    return op


def _f32(a):
    return a.astype(np.float32).reshape(a.shape[0], -1)


def _ref_relu2(in0, in1, c0, c1, c2):
    s = np.maximum(np.nan_to_num(_f32(in0) * c0 + _f32(in1) * c1,
                                 nan=0.0, posinf=np.inf, neginf=-np.inf), 0)
    return s.reshape(in0.shape)


RELU2 = _register_dve("RELU2_BIO", _dve_relu_expr(Src0 * C0 + Src1 * C1),
                      _ref_relu2, rd1=True)

F32 = mybir.dt.float32
F16 = mybir.dt.float16
RELU_FN = mybir.ActivationFunctionType.Relu

R = 512          # n_rec
NIN = 128        # n_in
RC = 4           # r chunks
B = 64           # batch (full, per window)
N_CORES = 8
NV = 2           # windows per core
NWIN = N_CORES * NV
T_FULL = 1000
BURN = int(os.environ.get("BIO_BURN", "48"))  # burn-in steps
WSTARTS = [(w * T_FULL) // NWIN for w in range(NWIN + 1)]
WLEN = max(WSTARTS[w + 1] - WSTARTS[w] for w in range(NWIN))  # 63
T_LOC = WLEN + BURN       # local steps per window
ALPHA = 0.2
LEAK = 1.0 - ALPHA
Q = 40                    # accum-q rescale block (0.8^-39 ~ 6e3: fp16-safe)
QOFF = (0, Q // 2)        # per-window frame offset: restarts never coincide
U = 64                    # r' ring steps
CPS = NV * 256            # ring cols per step-slot
ZR = 24                   # z-stream ring steps: the ring WAR paces the
                          # in-DMA against compute (else gpsimd floods
                          # HBM with the whole stream and PE inst-fetch
                          # starves)
ZCH = 8                   # z-stream DMA chunk (steps)
ZLEAD = 16                # z chunks DMA'd this many steps ahead
OCH = 4                   # out-dump chunk (steps; >0.5MB transfers
                          # monopolize HBM and starve PE inst-fetch)
OSKIP = BURN - 24         # dump starts here: 24 pre-output steps suffice
                          # for host h-integration (0.8^24 ~ 5e-3 missing)


def build_nc(T=T_LOC):
    """Build the per-core Bass program."""
    nc = bacc.Bacc()

    # host-precomputed pre-scaled z streams, fp16, transposed layout
    # [p, v, t, c]: c = m*64+b; c 0:128 = delta-enc (bank A), 128:256 = raw
    z_d = nc.dram_tensor("z16", [128, NV, T, 256], F16,
                         kind="ExternalInput").ap()
    w_d = nc.dram_tensor("w16", [R, R], F16, kind="ExternalInput").ap()
    # raw r' dump (steps OSKIP..T): [p, t, v*256 + m*64 + b] fp16;
    # host integrates h
    o_d = nc.dram_tensor("outT16", [128, T - OSKIP, CPS], F16,
                         kind="ExternalOutput").ap()

    with tile.TileContext(nc) as tc, ExitStack() as ctx:
        const = ctx.enter_context(tc.tile_pool(name="const", bufs=1))
        big = ctx.enter_context(tc.tile_pool(name="big", bufs=1))
        sp = ctx.enter_context(tc.tile_pool(name="sp", bufs=2))

        ident16 = const.tile([128, 128], F16)
        make_identity(nc, ident16[:, :])
        zero16 = const.tile([128, 128], F16)
        nc.vector.memset(zero16[:, :], 0.0)

        w16 = big.tile([128, RC * R], F16)
        zt = big.tile([128, NV * ZR * 256], F16)  # z stream ring
        rr = big.tile([128, U * CPS], F16)        # r' ring

        pools = [ctx.enter_context(
            tc.tile_pool(name=f"ps{n}", bufs=1, space="PSUM"))
            for n in ("a0", "c0", "a1", "c1")]
        pst = [p.tile([128, 512], F32, name=f"ps{i}", tag=f"ps{i}")
               for i, p in enumerate(pools)]
        psA = [pst[0], pst[2]]
        psC = [pst[1], pst[3]]

        # ---- DMA helpers: in-triggers must never sit behind sem-gated
        # out-triggers (head-of-line), so the streams get separate queues
        def emit_in(v, t0, t1):
            t1 = min(t1, T)
            if t0 >= t1:
                return
            s0 = (v * ZR + t0 % ZR) * 256
            nc.gpsimd.dma_start(
                out=zt[:, s0:s0 + (t1 - t0) * 256]
                .rearrange("p (t c) -> p t c", c=256),
                in_=z_d[:, v, t0:t1, :])

        def emit_out(a, e):
            s0 = (a % U) * CPS
            nc.sync.dma_start(
                out=o_d[:, a - OSKIP:e - OSKIP, :],
                in_=rr[:, s0:s0 + (e - a) * CPS]
                .rearrange("p (t c) -> p t c", c=CPS))

        # startup: small first chunks land fast, weights on their own queue
        emit_in(0, 0, 4)
        nc.sync.dma_start(
            out=w16[:, :].rearrange("p (k c) -> p k c", c=R),
            in_=w_d.rearrange("(k p) c -> p k c", p=128))
        emit_in(1, 0, 4)
        emit_in(0, 4, 8)
        emit_in(1, 4, 8)
        for t0 in range(ZCH, min(ZR, T), ZCH):
            for v in range(NV):
                emit_in(v, t0, t0 + ZCH)

        mm = nc.tensor.matmul

        def block(v, t):
            pa, pc = psA[v], psC[v]
            base = (v * ZR + t % ZR) * 256
            zA = zt[:, base:base + 128]
            zC = zt[:, base + 128:base + 256]
            so = (t % U) * CPS + v * 256
            outA = rr[:, so:so + 128]
            outC = rr[:, so + 128:so + 256]
            pr = ((t - 1) % U) * CPS + v * 256

            def rk(k):
                return rr[:, pr + k * 64:pr + (k + 1) * 64]

            if t == 0:
                mm(pa[:, :128], lhsT=ident16[:, :], rhs=zA,
                   start=True, stop=True, skip_group_check=True)
                mm(pc[:, :128], lhsT=ident16[:, :], rhs=zero16[:, :],
                   start=True, stop=True, skip_group_check=True)
            else:
                if (t + QOFF[v]) % Q == 0:
                    # re-inject q at true scale (q := 0.8^Q * q)
                    s16a = sp.tile([128, 128], F16, tag=f"s16a{v}")
                    s16c = sp.tile([128, 128], F16, tag=f"s16c{v}")
                    nc.scalar.mul(out=s16a[:, :], in_=pa[:, :128],
                                  mul=float(LEAK ** Q))
                    nc.vector.tensor_scalar_mul(s16c[:, :], pc[:, :128],
                                                float(LEAK ** Q))
                    mm(pa[:, :128], lhsT=ident16[:, :], rhs=s16a[:, :],
                       start=True, stop=False, skip_group_check=True)
                    mm(pc[:, :128], lhsT=ident16[:, :], rhs=s16c[:, :],
                       start=True, stop=False, skip_group_check=True)

                def kmm(m, k, stop=False):
                    ps = pa if m < 2 else pc
                    off = (m % 2) * 64
                    mm(ps[:, off:off + 64],
                       lhsT=w16[:, k * R + m * 128:k * R + (m + 1) * 128],
                       rhs=rk(k), start=False, stop=stop,
                       skip_group_check=True)

                mm(pa[:, :128], lhsT=ident16[:, :], rhs=zA,
                   start=False, stop=False, skip_group_check=True)
                kmm(0, 0); kmm(1, 0); kmm(0, 1); kmm(1, 1)
                kmm(2, 0); kmm(3, 0); kmm(2, 1); kmm(3, 1)
                kmm(0, 2); kmm(1, 2); kmm(0, 3); kmm(1, 3, stop=True)
                kmm(2, 2); kmm(3, 2); kmm(2, 3); kmm(3, 3, stop=True)

            jp = (t + QOFF[v]) % Q
            jn = (t + 1 + QOFF[v]) % Q
            c0 = float(ALPHA * LEAK ** (jp - jn))
            nc.scalar.activation(out=outA, in_=pa[:, :128], func=RELU_FN,
                                 scale=c0)
            nc.vector._custom_dve(RELU2, out=outC, in0=pc[:, :128],
                                  in1=zC, s0=c0, s1=c0)

        for t in range(T):
            tp = t + ZLEAD
            if tp % ZCH == 0 and ZR <= tp < T:
                emit_in(0, tp, tp + ZCH)
                emit_in(1, tp, tp + ZCH)
            block(0, t)
            block(1, t)
            if (t + 1 - OSKIP) % OCH == 0 or t == T - 1:
                a = OSKIP + ((t - OSKIP) // OCH) * OCH
                if a >= OSKIP:
                    emit_out(a, t + 1)

    nc.compile()
    return nc


def host_prep(x, w_in, w_rec, b_rec, ei_mask, autapse_mask, noise):
    """Host-side weight prep + window shard + pre-scaled fp16 z streams.

    z = x@w_in + noise + b_rec. Bank-A half (r 0:256): delta-encoded
    (z_t - 0.8 z_{t-1}) for psum injection. Bank-C half (r 256:512):
    raw (added at the DVE relu). Both scaled by 0.8^-(t % Q).
    """
    ei = np.diagonal(np.asarray(ei_mask)).astype(np.float32)
    w_eff = ei[:, None] * (np.asarray(w_rec) * np.asarray(autapse_mask))
    w16 = w_eff.astype(np.float16)
    x = np.asarray(x, dtype=np.float32)
    z = (x.reshape(-1, NIN) @ np.asarray(w_in, dtype=np.float32)).reshape(
        B, T_FULL, R)
    z += np.asarray(noise, dtype=np.float32)
    z += np.asarray(b_rec, np.float32)
    in_maps = []
    for c in range(N_CORES):
        zwins = []
        for v in range(NV):
            jscale = (LEAK ** -((np.arange(T_LOC) + QOFF[v]) % Q)
                      ).astype(np.float32)
            w = NV * c + v
            t0 = WSTARTS[w] - BURN
            zp = np.zeros((B, T_LOC, R), np.float32)
            s = max(t0, 0)
            zp[:, s - t0:] = z[:, s:t0 + T_LOC]
            # [p, t, m, b]
            zt4 = zp.reshape(B, T_LOC, RC, 128).transpose(3, 1, 2, 0)
            dA = zt4[:, :, 0:2, :].copy()
            dA[:, 1:] -= LEAK * dA[:, :-1].copy()
            dA *= jscale[None, :, None, None]
            zC = zt4[:, :, 2:4, :] * jscale[None, :, None, None]
            zwins.append(np.concatenate(
                [dA.reshape(128, T_LOC, 128), zC.reshape(128, T_LOC, 128)],
                axis=2))
        z16 = np.ascontiguousarray(
            np.stack(zwins, axis=1).astype(np.float16))
        in_maps.append({"z16": z16, "w16": w16})
    return in_maps, w_eff.astype(np.float32)


def _integrate(dump):
    """dump: [128, T_LOC-OSKIP, CPS] fp16 per core (steps OSKIP..T_LOC)
    -> list of NV h arrays (B, T_LOC-OSKIP, R) f32 via host leaky
    integration from zero at OSKIP."""
    td = dump.shape[1]
    hs = []
    for v in range(NV):
        jn = ((np.arange(td) + OSKIP + 1 + QOFF[v]) % Q).astype(np.float32)
        sc = (LEAK ** jn).astype(np.float32)
        rp = dump[:, :, v * 256:(v + 1) * 256].astype(np.float32)
        # [p, t, m, b] -> [b, t, r]
        ar = rp.reshape(128, td, RC, B).transpose(3, 1, 2, 0).reshape(
            B, td, R)
        ar *= sc[None, :, None]
        h = np.empty_like(ar)
        acc = np.zeros((B, R), np.float32)
        for t in range(td):
            acc = LEAK * acc + ar[:, t]
            h[:, t] = acc
        hs.append(h)
    return hs


def reference_np(x, w_in, b_rec, w_eff, noise, T=None):
    """Numpy reference for dev checks (f32)."""
    x = np.asarray(x, np.float32)
    if T is None:
        T = x.shape[1]
    z = np.einsum("bti,ir->btr", x[:, :T], np.asarray(w_in)) \
        + np.asarray(noise)[:, :T] + np.asarray(b_rec)
    h = np.zeros((x.shape[0], w_eff.shape[0]), np.float32)
    outs = []
    for t in range(T):
        pre = z[:, t] + h @ w_eff
        h = LEAK * h + ALPHA * np.maximum(pre, 0.0)
        outs.append(h.copy())
    return np.stack(outs, axis=1)


# ---------------------------------------------------------------------------
# harness entry point
# ---------------------------------------------------------------------------
_NC_CACHE = {}


def kernel(x, w_in, w_rec, b_rec, ei_mask, autapse_mask, noise):
    from concourse.bass_utils import run_bass_kernel_spmd

    x = np.asarray(x)
    T = x.shape[1]
    in_maps, _ = host_prep(x, w_in, w_rec, b_rec, ei_mask, autapse_mask, noise)
    if T not in _NC_CACHE:
        _NC_CACHE[T] = build_nc()
    nc = _NC_CACHE[T]
    res = run_bass_kernel_spmd(nc, in_maps, core_ids=list(range(N_CORES)))
    out = np.empty((x.shape[0], T, R), np.float32)
    for c in range(N_CORES):
        hs = _integrate(res.results[c]["outT16"])
        for v in range(NV):
            w = NV * c + v
            a, e = WSTARTS[w], WSTARTS[w + 1]
            b0 = BURN - OSKIP
            out[:, a:e] = hs[v][:, b0:b0 + (e - a)]
    return out


# revision 30
# speedup vs baseline: 1.1871x; 1.0237x over previous
"""BioRNN Trainium2 kernel (dev module).

Sharding: 16 time-windows (2 per core, ~63 output steps each), full
batch 64 per window. The leak (0.8/step) forgets initial state: a
32-step burn-in from h=0 reproduces the true state (window 0 pads with
zeros, exact). Per core: 2 windows x T_LOC=95 steps, emitted half-step
out of phase so one window's relu handoff hides under the other's
matmuls.

accum-q recurrence, v-part only in psum: q = 0.8^-j * v_t within a
Q=32 block (j = t % Q), v_t = h_{t-1} @ w_eff. Per window-step:
    1 inj matmul   identity @ dz_t (bank A only; N=128)
    16 W matmuls   r'_t @ w_eff (N=64)
where dz is the HOST-precomputed delta of z = x@w_in + noise + b_rec
for the bank-A half (r 0:256), pre-scaled by 0.8^-j, fp16, transposed.
Bank C's z half never enters psum: it rides in through a 2-input DVE
relu. The two relus run on DIFFERENT engines to halve the handoff
serialization:
    bank A (r 0:256):  ACT   r'a = relu(qA * c0)         (z in psum)
    bank C (r 256:512): DVE  r'c = relu(qC*c0 + z*c0)    (z from SBUF)
with c0 = 0.2*0.8^(jp-jn). Every Q steps each bank is re-injected at
true scale (ACT/DVE mul -> fp16 -> identity matmul, start=True).

h is NOT computed on device: h_t = 0.8 h_{t-1} + 0.8^jn r'_t is a
leaky integration the HOST does in f32 over the dumped r' stream
(more accurate than the old fp16 device ring). r' ring chunks DMA to
DRAM in native (p, t, v, m, b) fp16 layout; host un-transposes,
rescales by 0.8^jn(t), and lfilters to h.
"""

import os
import numpy as np
from contextlib import ExitStack

import concourse.bass as bass
import concourse.mybir as mybir
import concourse.tile as tile
from concourse import bacc
from concourse import dve_ops
from concourse.dve_spec import (
    Spec, Src0, Src1, C0, C1, relu as _dve_relu_expr, lower,
)
from concourse.dve_uop import DveOpSpec
from concourse.masks import make_identity


def _register_dve(name, body, ref, rd1=True):
    """Register a custom DVE op (idempotent)."""
    for o in dve_ops.OPS:
        if o.name == name:
            return o
    opcode = max(dve_ops._SUB_OPCODE_FOR_NAME.values()) + 1
    assert opcode < 0x20
    dve_ops._SUB_OPCODE_FOR_NAME[name] = opcode
    spec = Spec(body=body, reference=ref)
    shas = {}
    for ver in ("v3", "v4"):
        s = DveOpSpec(name=name, opcode=opcode, uops=lower(spec, ver=ver),
                      rd1_en=rd1)
        shas[ver] = s.sha(ver)
    op = dve_ops.DveOp(name, spec, subdim=False, uops_sha=shas)
    dve_ops.OPS.append(op)
    dve_ops.CUSTOM_DVE_SPECS[name] = spec
    return op


def _f32(a):
    return a.astype(np.float32).reshape(a.shape[0], -1)


def _ref_relu2(in0, in1, c0, c1, c2):
    s = np.maximum(np.nan_to_num(_f32(in0) * c0 + _f32(in1) * c1,
                                 nan=0.0, posinf=np.inf, neginf=-np.inf), 0)
    return s.reshape(in0.shape)


RELU2 = _register_dve("RELU2_BIO", _dve_relu_expr(Src0 * C0 + Src1 * C1),
                      _ref_relu2, rd1=True)

F32 = mybir.dt.float32
F16 = mybir.dt.float16
RELU_FN = mybir.ActivationFunctionType.Relu

R = 512          # n_rec
NIN = 128        # n_in
RC = 4           # r chunks
B = 64           # batch (full, per window)
N_CORES = 8
NV = 2           # windows per core
NWIN = N_CORES * NV
T_FULL = 1000
BURN = int(os.environ.get("BIO_BURN", "48"))  # burn-in steps
WSTARTS = [(w * T_FULL) // NWIN for w in range(NWIN + 1)]
WLEN = max(WSTARTS[w + 1] - WSTARTS[w] for w in range(NWIN))  # 63
T_LOC = WLEN + BURN       # local steps per window
ALPHA = 0.2
LEAK = 1.0 - ALPHA
Q = 40                    # accum-q rescale block (0.8^-39 ~ 6e3: fp16-safe)
QOFF = (0, Q // 2)        # per-window frame offset: restarts never coincide
U = 64                    # r' ring steps
CPS = NV * 256            # ring cols per step-slot
ZR = 24                   # z-stream ring steps: the ring WAR paces the
                          # in-DMA against compute (else gpsimd floods
                          # HBM with the whole stream and PE inst-fetch
                          # starves)
ZCH = 4                   # z-stream DMA chunk (steps)
ZLEAD = 14                # z lead; ≡2 mod 4 so in-chunks interleave
                          # between out-chunks instead of colliding
OCH = 4                   # out-dump chunk (steps; >0.5MB transfers
                          # monopolize HBM and starve PE inst-fetch)
OSKIP = BURN - 24         # dump starts here: 24 pre-output steps suffice
                          # for host h-integration (0.8^24 ~ 5e-3 missing)


def build_nc(T=T_LOC):
    """Build the per-core Bass program."""
    nc = bacc.Bacc()

    # host-precomputed pre-scaled z streams, fp16, transposed layout
    # [p, v, t, c]: c = m*64+b; c 0:128 = delta-enc (bank A), 128:256 = raw
    z_d = nc.dram_tensor("z16", [128, NV, T, 256], F16,
                         kind="ExternalInput").ap()
    w_d = nc.dram_tensor("w16", [R, R], F16, kind="ExternalInput").ap()
    # raw r' dump (steps OSKIP..T): [p, t, v*256 + m*64 + b] fp16;
    # host integrates h
    o_d = nc.dram_tensor("outT16", [128, T - OSKIP, CPS], F16,
                         kind="ExternalOutput").ap()

    with tile.TileContext(nc) as tc, ExitStack() as ctx:
        const = ctx.enter_context(tc.tile_pool(name="const", bufs=1))
        big = ctx.enter_context(tc.tile_pool(name="big", bufs=1))
        sp = ctx.enter_context(tc.tile_pool(name="sp", bufs=2))

        ident16 = const.tile([128, 128], F16)
        make_identity(nc, ident16[:, :])
        zero16 = const.tile([128, 128], F16)
        nc.vector.memset(zero16[:, :], 0.0)

        w16 = big.tile([128, RC * R], F16)
        zt = big.tile([128, NV * ZR * 256], F16)  # z stream ring
        rr = big.tile([128, U * CPS], F16)        # r' ring

        pools = [ctx.enter_context(
            tc.tile_pool(name=f"ps{n}", bufs=1, space="PSUM"))
            for n in ("a0", "c0", "a1", "c1")]
        pst = [p.tile([128, 512], F32, name=f"ps{i}", tag=f"ps{i}")
               for i, p in enumerate(pools)]
        psA = [pst[0], pst[2]]
        psC = [pst[1], pst[3]]

        # ---- DMA helpers: in-triggers must never sit behind sem-gated
        # out-triggers (head-of-line), so the streams get separate queues
        def emit_in(v, t0, t1):
            t1 = min(t1, T)
            if t0 >= t1:
                return
            s0 = (v * ZR + t0 % ZR) * 256
            nc.gpsimd.dma_start(
                out=zt[:, s0:s0 + (t1 - t0) * 256]
                .rearrange("p (t c) -> p t c", c=256),
                in_=z_d[:, v, t0:t1, :])

        def emit_out(a, e):
            s0 = (a % U) * CPS
            nc.sync.dma_start(
                out=o_d[:, a - OSKIP:e - OSKIP, :],
                in_=rr[:, s0:s0 + (e - a) * CPS]
                .rearrange("p (t c) -> p t c", c=CPS))

        # startup: both windows' first chunks race on separate queues,
        # then weights; ring fill alternates queues
        nc.gpsimd.dma_start(
            out=zt[:, 0:4 * 256].rearrange("p (t c) -> p t c", c=256),
            in_=z_d[:, 0, 0:4, :])
        nc.sync.dma_start(
            out=zt[:, ZR * 256:(ZR + 4) * 256]
            .rearrange("p (t c) -> p t c", c=256),
            in_=z_d[:, 1, 0:4, :])
        nc.sync.dma_start(
            out=w16[:, :].rearrange("p (k c) -> p k c", c=R),
            in_=w_d.rearrange("(k p) c -> p k c", p=128))
        emit_in(0, 4, 8)
        emit_in(1, 4, 8)
        for t0 in range(ZCH, min(ZR, T), ZCH):
            if t0 == 4:
                continue
            for v in range(NV):
                emit_in(v, t0, t0 + ZCH)

        mm = nc.tensor.matmul

        def block(v, t):
            pa, pc = psA[v], psC[v]
            base = (v * ZR + t % ZR) * 256
            zA = zt[:, base:base + 128]
            zC = zt[:, base + 128:base + 256]
            so = (t % U) * CPS + v * 256
            outA = rr[:, so:so + 128]
            outC = rr[:, so + 128:so + 256]
            pr = ((t - 1) % U) * CPS + v * 256

            def rk(k):
                return rr[:, pr + k * 64:pr + (k + 1) * 64]

            if t == 0:
                mm(pa[:, :128], lhsT=ident16[:, :], rhs=zA,
                   start=True, stop=True, skip_group_check=True)
                mm(pc[:, :128], lhsT=ident16[:, :], rhs=zero16[:, :],
                   start=True, stop=True, skip_group_check=True)
            else:
                if (t + QOFF[v]) % Q == 0:
                    # re-inject q at true scale (q := 0.8^Q * q)
                    s16a = sp.tile([128, 128], F16, tag=f"s16a{v}")
                    s16c = sp.tile([128, 128], F16, tag=f"s16c{v}")
                    nc.scalar.mul(out=s16a[:, :], in_=pa[:, :128],
                                  mul=float(LEAK ** Q))
                    nc.vector.tensor_scalar_mul(s16c[:, :], pc[:, :128],
                                                float(LEAK ** Q))
                    mm(pa[:, :128], lhsT=ident16[:, :], rhs=s16a[:, :],
                       start=True, stop=False, skip_group_check=True)
                    mm(pc[:, :128], lhsT=ident16[:, :], rhs=s16c[:, :],
                       start=True, stop=False, skip_group_check=True)

                def kmm(m, k, stop=False):
                    ps = pa if m < 2 else pc
                    off = (m % 2) * 64
                    mm(ps[:, off:off + 64],
                       lhsT=w16[:, k * R + m * 128:k * R + (m + 1) * 128],
                       rhs=rk(k), start=False, stop=stop,
                       skip_group_check=True)

                mm(pa[:, :128], lhsT=ident16[:, :], rhs=zA,
                   start=False, stop=False, skip_group_check=True)
                kmm(0, 0); kmm(1, 0); kmm(0, 1); kmm(1, 1)
                kmm(2, 0); kmm(3, 0); kmm(2, 1); kmm(3, 1)
                kmm(0, 2); kmm(1, 2); kmm(0, 3); kmm(1, 3, stop=True)
                kmm(2, 2); kmm(3, 2); kmm(2, 3); kmm(3, 3, stop=True)

            jp = (t + QOFF[v]) % Q
            jn = (t + 1 + QOFF[v]) % Q
            c0 = float(ALPHA * LEAK ** (jp - jn))
            nc.scalar.activation(out=outA, in_=pa[:, :128], func=RELU_FN,
                                 scale=c0)
            nc.vector._custom_dve(RELU2, out=outC, in0=pc[:, :128],
                                  in1=zC, s0=c0, s1=c0)

        for t in range(T):
            tp = t + ZLEAD
            if tp % ZCH == 0 and ZR <= tp < T:
                emit_in(0, tp, tp + ZCH)
                emit_in(1, tp, tp + ZCH)
            block(0, t)
            block(1, t)
            if (t + 1 - OSKIP) % OCH == 0 or t == T - 1:
                a = OSKIP + ((t - OSKIP) // OCH) * OCH
                if a >= OSKIP:
                    emit_out(a, t + 1)

    nc.compile()
    return nc


def host_prep(x, w_in, w_rec, b_rec, ei_mask, autapse_mask, noise):
    """Host-side weight prep + window shard + pre-scaled fp16 z streams.

    z = x@w_in + noise + b_rec. Bank-A half (r 0:256): delta-encoded
    (z_t - 0.8 z_{t-1}) for psum injection. Bank-C half (r 256:512):
    raw (added at the DVE relu). Both scaled by 0.8^-(t % Q).
    """
    ei = np.diagonal(np.asarray(ei_mask)).astype(np.float32)
    w_eff = ei[:, None] * (np.asarray(w_rec) * np.asarray(autapse_mask))
    w16 = w_eff.astype(np.float16)
    x = np.asarray(x, dtype=np.float32)
    z = (x.reshape(-1, NIN) @ np.asarray(w_in, dtype=np.float32)).reshape(
        B, T_FULL, R)
    z += np.asarray(noise, dtype=np.float32)
    z += np.asarray(b_rec, np.float32)
    in_maps = []
    for c in range(N_CORES):
        zwins = []
        for v in range(NV):
            jscale = (LEAK ** -((np.arange(T_LOC) + QOFF[v]) % Q)
                      ).astype(np.float32)
            w = NV * c + v
            t0 = WSTARTS[w] - BURN
            zp = np.zeros((B, T_LOC, R), np.float32)
            s = max(t0, 0)
            zp[:, s - t0:] = z[:, s:t0 + T_LOC]
            # [p, t, m, b]
            zt4 = zp.reshape(B, T_LOC, RC, 128).transpose(3, 1, 2, 0)
            dA = zt4[:, :, 0:2, :].copy()
            dA[:, 1:] -= LEAK * dA[:, :-1].copy()
            dA *= jscale[None, :, None, None]
            zC = zt4[:, :, 2:4, :] * jscale[None, :, None, None]
            zwins.append(np.concatenate(
                [dA.reshape(128, T_LOC, 128), zC.reshape(128, T_LOC, 128)],
                axis=2))
        z16 = np.ascontiguousarray(
            np.stack(zwins, axis=1).astype(np.float16))
        in_maps.append({"z16": z16, "w16": w16})
    return in_maps, w_eff.astype(np.float32)


def _integrate(dump):
    """dump: [128, T_LOC-OSKIP, CPS] fp16 per core (steps OSKIP..T_LOC)
    -> list of NV h arrays (B, T_LOC-OSKIP, R) f32 via host leaky
    integration from zero at OSKIP."""
    td = dump.shape[1]
    hs = []
    for v in range(NV):
        jn = ((np.arange(td) + OSKIP + 1 + QOFF[v]) % Q).astype(np.float32)
        sc = (LEAK ** jn).astype(np.float32)
        rp = dump[:, :, v * 256:(v + 1) * 256].astype(np.float32)
        # [p, t, m, b] -> [b, t, r]
        ar = rp.reshape(128, td, RC, B).transpose(3, 1, 2, 0).reshape(
            B, td, R)
        ar *= sc[None, :, None]
        h = np.empty_like(ar)
        acc = np.zeros((B, R), np.float32)
        for t in range(td):
            acc = LEAK * acc + ar[:, t]
            h[:, t] = acc
        hs.append(h)
    return hs


def reference_np(x, w_in, b_rec, w_eff, noise, T=None):
    """Numpy reference for dev checks (f32)."""
    x = np.asarray(x, np.float32)
    if T is None:
        T = x.shape[1]
    z = np.einsum("bti,ir->btr", x[:, :T], np.asarray(w_in)) \
        + np.asarray(noise)[:, :T] + np.asarray(b_rec)
    h = np.zeros((x.shape[0], w_eff.shape[0]), np.float32)
    outs = []
    for t in range(T):
        pre = z[:, t] + h @ w_eff
        h = LEAK * h + ALPHA * np.maximum(pre, 0.0)
        outs.append(h.copy())
    return np.stack(outs, axis=1)


# ---------------------------------------------------------------------------
# harness entry point
# ---------------------------------------------------------------------------
_NC_CACHE = {}


def kernel(x, w_in, w_rec, b_rec, ei_mask, autapse_mask, noise):
    from concourse.bass_utils import run_bass_kernel_spmd

    x = np.asarray(x)
    T = x.shape[1]
    in_maps, _ = host_prep(x, w_in, w_rec, b_rec, ei_mask, autapse_mask, noise)
    if T not in _NC_CACHE:
        _NC_CACHE[T] = build_nc()
    nc = _NC_CACHE[T]
    res = run_bass_kernel_spmd(nc, in_maps, core_ids=list(range(N_CORES)))
    out = np.empty((x.shape[0], T, R), np.float32)
    for c in range(N_CORES):
        hs = _integrate(res.results[c]["outT16"])
        for v in range(NV):
            w = NV * c + v
            a, e = WSTARTS[w], WSTARTS[w + 1]
            b0 = BURN - OSKIP
            out[:, a:e] = hs[v][:, b0:b0 + (e - a)]
    return out


# revision 31
# speedup vs baseline: 1.2188x; 1.0267x over previous
"""BioRNN Trainium2 kernel (dev module).

Sharding: 16 time-windows (2 per core, ~63 output steps each), full
batch 64 per window. The leak (0.8/step) forgets initial state: a
32-step burn-in from h=0 reproduces the true state (window 0 pads with
zeros, exact). Per core: 2 windows x T_LOC=95 steps, emitted half-step
out of phase so one window's relu handoff hides under the other's
matmuls.

accum-q recurrence, v-part only in psum: q = 0.8^-j * v_t within a
Q=32 block (j = t % Q), v_t = h_{t-1} @ w_eff. Per window-step:
    1 inj matmul   identity @ dz_t (bank A only; N=128)
    16 W matmuls   r'_t @ w_eff (N=64)
where dz is the HOST-precomputed delta of z = x@w_in + noise + b_rec
for the bank-A half (r 0:256), pre-scaled by 0.8^-j, fp16, transposed.
Bank C's z half never enters psum: it rides in through a 2-input DVE
relu. The two relus run on DIFFERENT engines to halve the handoff
serialization:
    bank A (r 0:256):  ACT   r'a = relu(qA * c0)         (z in psum)
    bank C (r 256:512): DVE  r'c = relu(qC*c0 + z*c0)    (z from SBUF)
with c0 = 0.2*0.8^(jp-jn). Every Q steps each bank is re-injected at
true scale (ACT/DVE mul -> fp16 -> identity matmul, start=True).

h is NOT computed on device: h_t = 0.8 h_{t-1} + 0.8^jn r'_t is a
leaky integration the HOST does in f32 over the dumped r' stream
(more accurate than the old fp16 device ring). r' ring chunks DMA to
DRAM in native (p, t, v, m, b) fp16 layout; host un-transposes,
rescales by 0.8^jn(t), and lfilters to h.
"""

import os
import numpy as np
from contextlib import ExitStack

import concourse.bass as bass
import concourse.mybir as mybir
import concourse.tile as tile
from concourse import bacc
from concourse import dve_ops
from concourse.dve_spec import (
    Spec, Src0, Src1, C0, C1, relu as _dve_relu_expr, lower,
)
from concourse.dve_uop import DveOpSpec
from concourse.masks import make_identity


def _register_dve(name, body, ref, rd1=True):
    """Register a custom DVE op (idempotent)."""
    for o in dve_ops.OPS:
        if o.name == name:
            return o
    opcode = max(dve_ops._SUB_OPCODE_FOR_NAME.values()) + 1
    assert opcode < 0x20
    dve_ops._SUB_OPCODE_FOR_NAME[name] = opcode
    spec = Spec(body=body, reference=ref)
    shas = {}
    for ver in ("v3", "v4"):
        s = DveOpSpec(name=name, opcode=opcode, uops=lower(spec, ver=ver),
                      rd1_en=rd1)
        shas[ver] = s.sha(ver)
    op = dve_ops.DveOp(name, spec, subdim=False, uops_sha=shas)
    dve_ops.OPS.append(op)
    dve_ops.CUSTOM_DVE_SPECS[name] = spec
    return op


def _f32(a):
    return a.astype(np.float32).reshape(a.shape[0], -1)


def _ref_relu2(in0, in1, c0, c1, c2):
    s = np.maximum(np.nan_to_num(_f32(in0) * c0 + _f32(in1) * c1,
                                 nan=0.0, posinf=np.inf, neginf=-np.inf), 0)
    return s.reshape(in0.shape)


RELU2 = _register_dve("RELU2_BIO", _dve_relu_expr(Src0 * C0 + Src1 * C1),
                      _ref_relu2, rd1=True)

F32 = mybir.dt.float32
F16 = mybir.dt.float16
RELU_FN = mybir.ActivationFunctionType.Relu

R = 512          # n_rec
NIN = 128        # n_in
RC = 4           # r chunks
B = 64           # batch (full, per window)
N_CORES = 8
NV = 2           # windows per core
NWIN = N_CORES * NV
T_FULL = 1000
BURN = int(os.environ.get("BIO_BURN", "44"))  # burn-in steps
WSTARTS = [(w * T_FULL) // NWIN for w in range(NWIN + 1)]
WLEN = max(WSTARTS[w + 1] - WSTARTS[w] for w in range(NWIN))  # 63
T_LOC = WLEN + BURN       # local steps per window
ALPHA = 0.2
LEAK = 1.0 - ALPHA
Q = 40                    # accum-q rescale block (0.8^-39 ~ 6e3: fp16-safe)
QOFF = (0, Q // 2)        # per-window frame offset: restarts never coincide
U = 64                    # r' ring steps
CPS = NV * 256            # ring cols per step-slot
ZR = 24                   # z-stream ring steps: the ring WAR paces the
                          # in-DMA against compute (else gpsimd floods
                          # HBM with the whole stream and PE inst-fetch
                          # starves)
ZCH = 4                   # z-stream DMA chunk (steps)
ZLEAD = 14                # z lead; ≡2 mod 4 so in-chunks interleave
                          # between out-chunks instead of colliding
OCH = 4                   # out-dump chunk (steps; >0.5MB transfers
                          # monopolize HBM and starve PE inst-fetch)
OSKIP = BURN - 24         # dump starts here: 24 pre-output steps suffice
                          # for host h-integration (0.8^24 ~ 5e-3 missing)


def build_nc(T=T_LOC):
    """Build the per-core Bass program."""
    nc = bacc.Bacc()

    # host-precomputed pre-scaled z streams, fp16, transposed layout
    # [p, v, t, c]: c = m*64+b; c 0:128 = delta-enc (bank A), 128:256 = raw
    z_d = nc.dram_tensor("z16", [128, NV, T, 256], F16,
                         kind="ExternalInput").ap()
    w_d = nc.dram_tensor("w16", [R, R], F16, kind="ExternalInput").ap()
    # raw r' dump (steps OSKIP..T): [p, t, v*256 + m*64 + b] fp16;
    # host integrates h
    o_d = nc.dram_tensor("outT16", [128, T - OSKIP, CPS], F16,
                         kind="ExternalOutput").ap()

    with tile.TileContext(nc) as tc, ExitStack() as ctx:
        const = ctx.enter_context(tc.tile_pool(name="const", bufs=1))
        big = ctx.enter_context(tc.tile_pool(name="big", bufs=1))
        sp = ctx.enter_context(tc.tile_pool(name="sp", bufs=2))

        ident16 = const.tile([128, 128], F16)
        make_identity(nc, ident16[:, :])
        zero16 = const.tile([128, 128], F16)
        nc.vector.memset(zero16[:, :], 0.0)

        w16 = big.tile([128, RC * R], F16)
        zt = big.tile([128, NV * ZR * 256], F16)  # z stream ring
        rr = big.tile([128, U * CPS], F16)        # r' ring

        pools = [ctx.enter_context(
            tc.tile_pool(name=f"ps{n}", bufs=1, space="PSUM"))
            for n in ("a0", "c0", "a1", "c1")]
        pst = [p.tile([128, 512], F32, name=f"ps{i}", tag=f"ps{i}")
               for i, p in enumerate(pools)]
        psA = [pst[0], pst[2]]
        psC = [pst[1], pst[3]]

        # ---- DMA helpers: in-triggers must never sit behind sem-gated
        # out-triggers (head-of-line), so the streams get separate queues
        def emit_in(v, t0, t1):
            t1 = min(t1, T)
            if t0 >= t1:
                return
            s0 = (v * ZR + t0 % ZR) * 256
            nc.gpsimd.dma_start(
                out=zt[:, s0:s0 + (t1 - t0) * 256]
                .rearrange("p (t c) -> p t c", c=256),
                in_=z_d[:, v, t0:t1, :])

        def emit_out(a, e):
            s0 = (a % U) * CPS
            nc.sync.dma_start(
                out=o_d[:, a - OSKIP:e - OSKIP, :],
                in_=rr[:, s0:s0 + (e - a) * CPS]
                .rearrange("p (t c) -> p t c", c=CPS))

        # startup: both windows' first chunks race on separate queues,
        # then weights; ring fill alternates queues
        nc.gpsimd.dma_start(
            out=zt[:, 0:4 * 256].rearrange("p (t c) -> p t c", c=256),
            in_=z_d[:, 0, 0:4, :])
        nc.sync.dma_start(
            out=zt[:, ZR * 256:(ZR + 4) * 256]
            .rearrange("p (t c) -> p t c", c=256),
            in_=z_d[:, 1, 0:4, :])
        nc.sync.dma_start(
            out=w16[:, :].rearrange("p (k c) -> p k c", c=R),
            in_=w_d.rearrange("(k p) c -> p k c", p=128))
        emit_in(0, 4, 8)
        emit_in(1, 4, 8)
        for t0 in range(ZCH, min(ZR, T), ZCH):
            if t0 == 4:
                continue
            for v in range(NV):
                emit_in(v, t0, t0 + ZCH)

        mm = nc.tensor.matmul

        def block(v, t):
            pa, pc = psA[v], psC[v]
            base = (v * ZR + t % ZR) * 256
            zA = zt[:, base:base + 128]
            zC = zt[:, base + 128:base + 256]
            so = (t % U) * CPS + v * 256
            outA = rr[:, so:so + 128]
            outC = rr[:, so + 128:so + 256]
            pr = ((t - 1) % U) * CPS + v * 256

            def rk(k):
                return rr[:, pr + k * 64:pr + (k + 1) * 64]

            if t == 0:
                mm(pa[:, :128], lhsT=ident16[:, :], rhs=zA,
                   start=True, stop=True, skip_group_check=True)
                mm(pc[:, :128], lhsT=ident16[:, :], rhs=zero16[:, :],
                   start=True, stop=True, skip_group_check=True)
            else:
                if (t + QOFF[v]) % Q == 0:
                    # re-inject q at true scale (q := 0.8^Q * q)
                    s16a = sp.tile([128, 128], F16, tag=f"s16a{v}")
                    s16c = sp.tile([128, 128], F16, tag=f"s16c{v}")
                    nc.scalar.mul(out=s16a[:, :], in_=pa[:, :128],
                                  mul=float(LEAK ** Q))
                    nc.vector.tensor_scalar_mul(s16c[:, :], pc[:, :128],
                                                float(LEAK ** Q))
                    mm(pa[:, :128], lhsT=ident16[:, :], rhs=s16a[:, :],
                       start=True, stop=False, skip_group_check=True)
                    mm(pc[:, :128], lhsT=ident16[:, :], rhs=s16c[:, :],
                       start=True, stop=False, skip_group_check=True)

                def kmm(m, k, stop=False):
                    ps = pa if m < 2 else pc
                    off = (m % 2) * 64
                    mm(ps[:, off:off + 64],
                       lhsT=w16[:, k * R + m * 128:k * R + (m + 1) * 128],
                       rhs=rk(k), start=False, stop=stop,
                       skip_group_check=True)

                mm(pa[:, :128], lhsT=ident16[:, :], rhs=zA,
                   start=False, stop=False, skip_group_check=True)
                kmm(0, 0); kmm(1, 0); kmm(0, 1); kmm(1, 1)
                kmm(2, 0); kmm(3, 0); kmm(2, 1); kmm(3, 1)
                kmm(0, 2); kmm(1, 2); kmm(0, 3); kmm(1, 3, stop=True)
                kmm(2, 2); kmm(3, 2); kmm(2, 3); kmm(3, 3, stop=True)

            jp = (t + QOFF[v]) % Q
            jn = (t + 1 + QOFF[v]) % Q
            c0 = float(ALPHA * LEAK ** (jp - jn))
            nc.scalar.activation(out=outA, in_=pa[:, :128], func=RELU_FN,
                                 scale=c0)
            nc.vector._custom_dve(RELU2, out=outC, in0=pc[:, :128],
                                  in1=zC, s0=c0, s1=c0)

        for t in range(T):
            tp = t + ZLEAD
            if tp % ZCH == 0 and ZR <= tp < T:
                emit_in(0, tp, tp + ZCH)
                emit_in(1, tp, tp + ZCH)
            block(0, t)
            block(1, t)
            if (t + 1 - OSKIP) % OCH == 0 or t == T - 1:
                a = OSKIP + ((t - OSKIP) // OCH) * OCH
                if a >= OSKIP:
                    emit_out(a, t + 1)

    nc.compile()
    return nc


def host_prep(x, w_in, w_rec, b_rec, ei_mask, autapse_mask, noise):
    """Host-side weight prep + window shard + pre-scaled fp16 z streams.

    z = x@w_in + noise + b_rec. Bank-A half (r 0:256): delta-encoded
    (z_t - 0.8 z_{t-1}) for psum injection. Bank-C half (r 256:512):
    raw (added at the DVE relu). Both scaled by 0.8^-(t % Q).
    """
    ei = np.diagonal(np.asarray(ei_mask)).astype(np.float32)
    w_eff = ei[:, None] * (np.asarray(w_rec) * np.asarray(autapse_mask))
    w16 = w_eff.astype(np.float16)
    x = np.asarray(x, dtype=np.float32)
    z = (x.reshape(-1, NIN) @ np.asarray(w_in, dtype=np.float32)).reshape(
        B, T_FULL, R)
    z += np.asarray(noise, dtype=np.float32)
    z += np.asarray(b_rec, np.float32)
    in_maps = []
    for c in range(N_CORES):
        zwins = []
        for v in range(NV):
            jscale = (LEAK ** -((np.arange(T_LOC) + QOFF[v]) % Q)
                      ).astype(np.float32)
            w = NV * c + v
            t0 = WSTARTS[w] - BURN
            zp = np.zeros((B, T_LOC, R), np.float32)
            s = max(t0, 0)
            zp[:, s - t0:] = z[:, s:t0 + T_LOC]
            # [p, t, m, b]
            zt4 = zp.reshape(B, T_LOC, RC, 128).transpose(3, 1, 2, 0)
            dA = zt4[:, :, 0:2, :].copy()
            dA[:, 1:] -= LEAK * dA[:, :-1].copy()
            dA *= jscale[None, :, None, None]
            zC = zt4[:, :, 2:4, :] * jscale[None, :, None, None]
            zwins.append(np.concatenate(
                [dA.reshape(128, T_LOC, 128), zC.reshape(128, T_LOC, 128)],
                axis=2))
        z16 = np.ascontiguousarray(
            np.stack(zwins, axis=1).astype(np.float16))
        in_maps.append({"z16": z16, "w16": w16})
    return in_maps, w_eff.astype(np.float32)


def _integrate(dump):
    """dump: [128, T_LOC-OSKIP, CPS] fp16 per core (steps OSKIP..T_LOC)
    -> list of NV h arrays (B, T_LOC-OSKIP, R) f32 via host leaky
    integration from zero at OSKIP."""
    td = dump.shape[1]
    hs = []
    for v in range(NV):
        jn = ((np.arange(td) + OSKIP + 1 + QOFF[v]) % Q).astype(np.float32)
        sc = (LEAK ** jn).astype(np.float32)
        rp = dump[:, :, v * 256:(v + 1) * 256].astype(np.float32)
        # [p, t, m, b] -> [b, t, r]
        ar = rp.reshape(128, td, RC, B).transpose(3, 1, 2, 0).reshape(
            B, td, R)
        ar *= sc[None, :, None]
        h = np.empty_like(ar)
        acc = np.zeros((B, R), np.float32)
        for t in range(td):
            acc = LEAK * acc + ar[:, t]
            h[:, t] = acc
        hs.append(h)
    return hs


def reference_np(x, w_in, b_rec, w_eff, noise, T=None):
    """Numpy reference for dev checks (f32)."""
    x = np.asarray(x, np.float32)
    if T is None:
        T = x.shape[1]
    z = np.einsum("bti,ir->btr", x[:, :T], np.asarray(w_in)) \
        + np.asarray(noise)[:, :T] + np.asarray(b_rec)
    h = np.zeros((x.shape[0], w_eff.shape[0]), np.float32)
    outs = []
    for t in range(T):
        pre = z[:, t] + h @ w_eff
        h = LEAK * h + ALPHA * np.maximum(pre, 0.0)
        outs.append(h.copy())
    return np.stack(outs, axis=1)


# ---------------------------------------------------------------------------
# harness entry point
# ---------------------------------------------------------------------------
_NC_CACHE = {}


def kernel(x, w_in, w_rec, b_rec, ei_mask, autapse_mask, noise):
    from concourse.bass_utils import run_bass_kernel_spmd

    x = np.asarray(x)
    T = x.shape[1]
    in_maps, _ = host_prep(x, w_in, w_rec, b_rec, ei_mask, autapse_mask, noise)
    if T not in _NC_CACHE:
        _NC_CACHE[T] = build_nc()
    nc = _NC_CACHE[T]
    res = run_bass_kernel_spmd(nc, in_maps, core_ids=list(range(N_CORES)))
    out = np.empty((x.shape[0], T, R), np.float32)
    for c in range(N_CORES):
        hs = _integrate(res.results[c]["outT16"])
        for v in range(NV):
            w = NV * c + v
            a, e = WSTARTS[w], WSTARTS[w + 1]
            b0 = BURN - OSKIP
            out[:, a:e] = hs[v][:, b0:b0 + (e - a)]
    return out
